# revision 21
# baseline (speedup 1.0000x reference)
# Trainium2 Bass kernel for nn_DSQN (2-layer LIF spiking MLP, snntorch Leaky
# subtract-reset semantics), batch-data-parallel across 8 NeuronCores.
#
# Math (per core, batch shard of 64):
#   mem_t = be*mem_{t-1} + cur_t - (mem_{t-1} > 1)    [per layer]
#   spk_t = (mem_t > 1)
#   cur1 = x @ W1.T + b1;  cur2 = spk1 @ W2.T + b2;  out = spk2 @ W3.T + b3
#
# v2 design notes (from HW probing):
#  - The per-step LIF recurrence is the critical path: one fused custom DVE
#    op per timestep ([128 hid, 2 layers, 64 batch]) runs at ~290ns/step when
#    its inputs are staged a full chunk ahead (vs ~446ns with same-iteration
#    staging).  The pipeline below computes cur1 for chunk it+1 while the
#    chain runs chunk it, so every chain step's operands are a chunk old.
#  - All matmuls use exact fp16 two-plane splits (probe: fp16 matmul is
#    bit-exact for fp16 inputs, single-pass ~216-375ns/512cols vs fp32's
#    2-pass ~1184ns).  W = U + V with U=fp16(W), V=fp16(W-U) carries 22
#    mantissa bits; x = A + B likewise.  L1 stacks the two x-planes on the
#    PE contraction axis (obs=64, so [A;B] fills 128 rows) with replicated
#    [U;U],[V;V] stationaries - 2 passes/psum tile, PSUM-accumulated.
#  - L3 packs its four [18,512] psum tiles into one bank at partition
#    offsets 0/32/64/96, so one scalar copy (with per-partition bias) moves
#    the whole chunk's output.
#  - Engine budget per 32-step chunk: DVE = 32 LIF + 1 spike-extract;
#    Scalar = 9 psum copies + 1 Sign; Tensor = 24 matmuls; GPSIMD unused
#    (probe: ~25x slower than DVE for bulk elementwise).

import numpy as np

import concourse.bacc as bacc
import concourse.bass as bass
import concourse.mybir as mybir
from concourse.tile import TileContext

F32 = mybir.dt.float32
F16 = mybir.dt.float16
OP = mybir.AluOpType
AF = mybir.ActivationFunctionType

B_TOTAL, T_FULL, OBS, HID, NACT = 512, 2048, 64, 128, 18
NCORES = 8
BC = B_TOTAL // NCORES      # 64 batch per core
TC = 32                     # time steps per chunk
SPT = 8                     # steps per psum tile (512 f32 columns)
NPT = TC // SPT             # 4 psum tiles per chunk


def _register_lif_op():
    """Register the fused LIF-step custom DVE op (runtime, idempotent).

    out[p, s, k] = in0[p, s, k] * (s0[p] if s == 0 else s1[p])
                   + in1[p, s*BC + k] - (in0[p, s, k] > 1.0)
    """
    import concourse.dve_ops as dve_ops
    for o in dve_ops.OPS:
        if o.name == "LIF_STEP_ANT":
            return o
    from concourse.dve_spec import (
        Spec, Src0, Src1, C0, C1, Zero, One, SubIdx, select, eq, lower,
        _has_src1)
    from concourse.dve_uop import DveOpSpec

    def _ref(in0, in1, s0, s1, imm2):
        m = np.asarray(in0, np.float32)
        c = np.asarray(in1, np.float32).reshape(m.shape)
        sc = (s0, s1)
        be = np.stack(
            [np.broadcast_to(
                np.reshape(np.asarray(sc[i], np.float32), (-1, 1)),
                (m.shape[0], m.shape[2]))
             for i in range(m.shape[1])], axis=1)
        r = (m > np.float32(1.0)).astype(np.float32)
        return (m * be + c) - r

    body = Src0 * select(eq(SubIdx, Zero), C0, C1) + Src1 - (Src0 > One)
    spec = Spec(body=body, reference=_ref)
    row = dve_ops._CUSTOM_DVE_ROW_BASE + len(dve_ops.OPS)
    shas = {}
    for ver in ("v3", "v4"):
        try:
            s = DveOpSpec(name="LIF_STEP_ANT", opcode=row,
                          uops=lower(spec, ver=ver), rd1_en=_has_src1(spec))
            shas[ver] = s.sha(ver)
        except Exception:
            pass
    op = dve_ops.DveOp("LIF_STEP_ANT", spec, subdim=True, uops_sha=shas)
    dve_ops.OPS.append(op)
    dve_ops.CUSTOM_DVE_SPECS[op.name] = op.spec
    dve_ops._SUB_OPCODE_FOR_NAME[op.name] = row
    return op


def _register_lif1_op():
    # Single-layer LIF op: single-FSM-state (no subdim/scan/select), so its
    # fixed cost is far lower than the fused two-layer op (dur 280 vs 410ns)
    # and the two per-layer ops of a step are mutually independent, so they
    # overlap on the DVE: 196ns/op cadence = 392ns/step vs 446 fused.
    import concourse.dve_ops as dve_ops
    from concourse.dve_spec import Spec, Src0, Src1, C0, One, lower, _has_src1
    from concourse.dve_uop import DveOpSpec
    for o in dve_ops.OPS:
        if o.name == "LIF1_ANT":
            return o

    def _ref(in0, in1, s0, s1, imm2):
        m = np.asarray(in0, np.float32)
        c = np.asarray(in1, np.float32).reshape(m.shape)
        be = np.reshape(np.asarray(s0, np.float32), (-1, 1))
        return (m * be + c) - (m > np.float32(1.0)).astype(np.float32)

    body = Src0 * C0 + Src1 - (Src0 > One)
    spec = Spec(body=body, reference=_ref)
    row = dve_ops._CUSTOM_DVE_ROW_BASE + len(dve_ops.OPS)
    shas = {}
    for ver in ("v3", "v4"):
        try:
            s = DveOpSpec(name="LIF1_ANT", opcode=row,
                          uops=lower(spec, ver=ver), rd1_en=_has_src1(spec))
            shas[ver] = s.sha(ver)
        except Exception:
            pass
    op = dve_ops.DveOp("LIF1_ANT", spec, subdim=False, uops_sha=shas)
    dve_ops.OPS.append(op)
    dve_ops.CUSTOM_DVE_SPECS[op.name] = op.spec
    dve_ops._SUB_OPCODE_FOR_NAME[op.name] = row
    return op


def build_nc(T=T_FULL):
    NCH = T // TC
    lif_op = _register_lif_op()
    lif1_op = _register_lif1_op()
    nc = bacc.Bacc()

    # x planes: rows 0-63 = fp16 hi plane of obs o, rows 64-127 = fp16 lo
    xt_d = nc.dram_tensor("xt", [HID, T, BC], F16, kind="ExternalInput")
    h0_d = nc.dram_tensor("h0", [BC, HID, 2], F32, kind="ExternalInput")
    # stationaries (fp16 two-plane splits, pre-stacked on host)
    s1a_d = nc.dram_tensor("s1a", [HID, HID], F16, kind="ExternalInput")
    s1b_d = nc.dram_tensor("s1b", [HID, HID], F16, kind="ExternalInput")
    s2a_d = nc.dram_tensor("s2a", [HID, HID], F16, kind="ExternalInput")
    s2b_d = nc.dram_tensor("s2b", [HID, HID], F16, kind="ExternalInput")
    s3a_d = nc.dram_tensor("s3a", [HID, NACT], F16, kind="ExternalInput")
    s3b_d = nc.dram_tensor("s3b", [HID, NACT], F16, kind="ExternalInput")
    b1_d = nc.dram_tensor("b1v", [HID, 1], F32, kind="ExternalInput")
    b2_d = nc.dram_tensor("b2v", [HID, 1], F32, kind="ExternalInput")
    b3_d = nc.dram_tensor("b3v", [HID, 1], F32, kind="ExternalInput")
    be1_d = nc.dram_tensor("be1", [HID, 1], F32, kind="ExternalInput")
    be2_d = nc.dram_tensor("be2", [HID, 1], F32, kind="ExternalInput")
    id_d = nc.dram_tensor("ident", [OBS, OBS], F32, kind="ExternalInput")
    # raw packed output: per chunk the [128, 512] ob tile (4 blocks of 18
    # used partitions at offsets 0/32/64/96); host unpacks
    y_d = nc.dram_tensor("yraw", [T // TC, NPT, NACT, SPT, BC], F32,
                         kind="ExternalOutput")

    with TileContext(nc) as tc:
        with (
            tc.tile_pool(name="const", bufs=1) as cpool,
            tc.tile_pool(name="state", bufs=1) as spool,
            tc.tile_pool(name="xt", bufs=3) as xt_pool,
            tc.tile_pool(name="mb", bufs=3) as mb_pool,
            tc.tile_pool(name="cb", bufs=4) as cb_pool,
            tc.tile_pool(name="spk1", bufs=2) as spk1_pool,
            tc.tile_pool(name="spk2", bufs=2) as spk2_pool,
            tc.tile_pool(name="ob", bufs=2) as ob_pool,
            tc.tile_pool(name="pc1", bufs=2, space=bass.MemorySpace.PSUM) as pc1_pool,
            tc.tile_pool(name="pc2", bufs=2, space=bass.MemorySpace.PSUM) as pc2_pool,
            tc.tile_pool(name="pout", bufs=2, space=bass.MemorySpace.PSUM) as po_pool,
        ):
            # ---- constants ----
            s1a = cpool.tile([HID, HID], F16, tag="s1a")
            s1b = cpool.tile([HID, HID], F16, tag="s1b")
            s2a = cpool.tile([HID, HID], F16, tag="s2a")
            s2b = cpool.tile([HID, HID], F16, tag="s2b")
            s3a = cpool.tile([HID, NACT], F16, tag="s3a")
            s3b = cpool.tile([HID, NACT], F16, tag="s3b")
            b1v = cpool.tile([HID, 1], F32, tag="b1v")
            b2v = cpool.tile([HID, 1], F32, tag="b2v")
            b3v = cpool.tile([HID, 1], F32, tag="b3v")
            be1 = cpool.tile([HID, 1], F32, tag="be1")
            be2 = cpool.tile([HID, 1], F32, tag="be2")
            ident = cpool.tile([OBS, OBS], F32, tag="ident")
            negone = cpool.tile([HID, 1], F32, tag="negone")
            nc.gpsimd.memset(negone[:], -1.0)

            mbufs, cbufs, spk1bufs, xtbufs = {}, {}, {}, {}

            def get_cb(n):
                if n not in cbufs:
                    cbufs[n] = cb_pool.tile([HID, TC, 2, BC], F32, tag="cb",
                                            name=f"cb{n}")
                return cbufs[n]

            def load_xt(n):
                if n not in xtbufs:
                    xt = xt_pool.tile([HID, TC, BC], F16, tag="xt",
                                      name=f"xt{n}")
                    nc.sync.dma_start(xt[:], xt_d[:, n * TC:(n + 1) * TC, :])
                    xtbufs[n] = xt
                return xtbufs[n]

            # critical-path DMAs first: everything the first chain chunk
            # needs (xt[0], h0, ident, L1 weights, chain constants) issues
            # before the bulk L2/L3 weights -- the sync queue is serial and
            # the first LIF waits on this whole path
            load_xt(0)
            h0ns = []
            for ch in range(2):
                h0n = spool.tile([BC, HID], F32, tag=f"h0n{ch}")
                nc.sync.dma_start(h0n[:], h0_d[:, :, ch])
                h0ns.append(h0n)
            for t_, d_ in ((ident, id_d), (s1a, s1a_d), (s1b, s1b_d),
                           (b1v, b1_d), (be1, be1_d), (be2, be2_d)):
                nc.sync.dma_start(t_[:], d_[:])

            # ---- initial membranes: [128 hid, 2 layer, 64 batch] ----
            minit = spool.tile([HID, 2, BC], F32, tag="minit")
            for ch in range(2):
                pt = pc1_pool.tile([HID, BC], F32, tag="c1ps", name=f"init{ch}")
                nc.tensor.transpose(pt[:], h0ns[ch][:], ident[:])
                nc.scalar.activation(minit[:, ch, :], pt[:], AF.Copy)

            # remaining weights (needed only from stage C / D onwards)
            for t_, d_ in ((b2v, b2_d), (s2a, s2a_d), (s2b, s2b_d),
                           (s3a, s3a_d), (s3b, s3b_d), (b3v, b3_d)):
                nc.sync.dma_start(t_[:], d_[:])

            for it in range(-1, NCH + 2):
                l1p = it + 1 if 0 <= it + 1 < NCH else None   # L1 prefetch
                l1 = it if 0 <= it < NCH else None            # layer1 of chain
                l2 = it - 2 if 0 <= it - 2 < NCH else None    # layer2 of chain
                jc2 = it - 1 if 0 <= it - 1 < NCH else None   # L2 mm source

                # ---- xt DMA two chunks ahead ----
                if 0 <= it + 2 < NCH:
                    load_xt(it + 2)

                # ---- stage A: cur1 for chunk it+1 (one chunk ahead) ----
                if l1p is not None:
                    xt = load_xt(l1p)
                    c1b = get_cb(l1p)
                    for g in range(NPT):
                        pc = pc1_pool.tile([HID, SPT, BC], F32, tag="c1ps")
                        nc.tensor.matmul(
                            pc[:], s1a[:], xt[:, g * SPT:(g + 1) * SPT, :],
                            start=True, stop=False)
                        nc.tensor.matmul(
                            pc[:], s1b[:], xt[:, g * SPT:(g + 1) * SPT, :],
                            start=False, stop=True)
                        nc.scalar.activation(
                            c1b[:, g * SPT:(g + 1) * SPT, 0, :], pc[:],
                            AF.Identity, bias=b1v[:])

                # ---- stage B: fused LIF chain for chunk it ----
                if l1 is not None or l2 is not None:
                    mb = mb_pool.tile([HID, TC, 2, BC], F32, tag="mb",
                                      name=f"mb{it}")
                    mbufs[it] = mb
                    cb = get_cb(it)
                    for s in range(TC):
                        if s == 0:
                            pm = mbufs.get(it - 1)
                            prev = minit[:] if pm is None else pm[:, TC - 1]
                        else:
                            prev = mb[:, s - 1]
                        if l1 is not None:
                            nc.vector._custom_dve(
                                lif1_op, out=mb[:, s, 0, :],
                                in0=prev[:, 0, :], in1=cb[:, s, 0, :],
                                s0=be1[:], s1=0.0)
                        if l2 is not None:
                            nc.vector._custom_dve(
                                lif1_op, out=mb[:, s, 1, :],
                                in0=prev[:, 1, :], in1=cb[:, s, 1, :],
                                s0=be2[:], s1=0.0)
                    if it == 1:  # seed layer-2 initial membrane for handoff
                        nc.vector.tensor_copy(mb[:, TC - 1, 1, :],
                                              minit[:, 1, :])
                    # spk1 of chunk it: sign(m-1) in {-1,+1} fp16 on the
                    # Scalar engine (keeps the DVE a pure LIF chain); W2 is
                    # halved on host and b2 absorbs +0.5*rowsum(W2)
                    if l1 is not None:
                        spk1 = spk1_pool.tile([HID, TC, BC], F16, tag="spk1",
                                              name=f"s1_{it}")
                        spk1bufs[it] = spk1
                        nc.scalar.activation(
                            spk1[:], mb[:, :, 0, :], AF.Sign, bias=negone[:])

                # ---- stage C: cur2 for chunk jc2 from spk1[jc2] ----
                if jc2 is not None:
                    c2dst = get_cb(jc2 + 2)
                    sm = spk1bufs[jc2]
                    for g in range(NPT):
                        pc = pc2_pool.tile([HID, SPT, BC], F32, tag="c2ps")
                        nc.tensor.matmul(
                            pc[:], s2a[:], sm[:, g * SPT:(g + 1) * SPT, :],
                            start=True, stop=False)
                        nc.tensor.matmul(
                            pc[:], s2b[:], sm[:, g * SPT:(g + 1) * SPT, :],
                            start=False, stop=True)
                        nc.scalar.activation(
                            c2dst[:, g * SPT:(g + 1) * SPT, 1, :], pc[:],
                            AF.Identity, bias=b2v[:])

                # ---- stage D: out for chunk l2 (sign spikes, packed psum) --
                if l2 is not None:
                    # spk2 as sign(m-1) in {-1,+1} fp16 (Scalar engine);
                    # W3 halved on host, bias3 absorbs +0.5*rowsum(W3)
                    spk2 = spk2_pool.tile([HID, TC, BC], F16, tag="spk2",
                                          name=f"s2_{it}")
                    nc.scalar.activation(
                        spk2[:], mbufs[it][:, :, 1, :], AF.Sign,
                        bias=negone[:])
                    # PE out base partition must be 0/32/64 (quadrant-3 bug):
                    # pack psum tiles 3+1 across two banks
                    po = po_pool.tile([HID, SPT, BC], F32, tag="ops")
                    pob = po_pool.tile([NACT, SPT, BC], F32, tag="opsb")
                    for g in range(NPT):
                        dst = po[32 * g:32 * g + NACT] if g < 3 else pob[:]
                        nc.tensor.matmul(
                            dst, s3a[:], spk2[:, g * SPT:(g + 1) * SPT, :],
                            start=True, stop=False)
                        nc.tensor.matmul(
                            dst, s3b[:], spk2[:, g * SPT:(g + 1) * SPT, :],
                            start=False, stop=True)
                    ob = ob_pool.tile([HID, SPT, BC], F32, tag="ob")
                    obb = ob_pool.tile([NACT, SPT, BC], F32, tag="obb")
                    nc.scalar.activation(ob[:], po[:], AF.Identity,
                                         bias=b3v[:])
                    nc.scalar.activation(obb[:], pob[:], AF.Identity,
                                         bias=b3v[0:NACT])
                    for g in range(3):
                        nc.sync.dma_start(y_d[l2, g],
                                          ob[32 * g:32 * g + NACT])
                    nc.sync.dma_start(y_d[l2, 3], obb[:])

    nc.compile()
    return nc


def _fp16_split(w):
    """w (fp32) -> (hi, lo) fp16 planes with hi+lo capturing 22 bits."""
    hi = w.astype(np.float16)
    lo = (w - hi.astype(np.float32)).astype(np.float16)
    return hi, lo


def _prep_core_inputs(inputs):
    x = np.ascontiguousarray(np.asarray(inputs["state_batch"], np.float32))
    h0 = np.asarray(inputs["hidden_states"], np.float32)[:, 0, :, :]
    W1 = np.asarray(inputs["W1"], np.float32)
    W2 = np.asarray(inputs["W2"], np.float32)
    W3 = np.asarray(inputs["W3"], np.float32)
    b1 = np.asarray(inputs["b1"], np.float32)
    b2 = np.asarray(inputs["b2"], np.float32)
    b3 = np.asarray(inputs["b3"], np.float32)
    be1 = np.clip(np.asarray(inputs["beta1"], np.float32), 0.0, 1.0)
    be2 = np.clip(np.asarray(inputs["beta2"], np.float32), 0.0, 1.0)

    u1, v1 = _fp16_split(W1.T)                    # [64, 128]
    u2, v2 = _fp16_split((0.5 * W2).T)            # [128, 128]
    u3, v3 = _fp16_split((0.5 * W3).T)            # [128, 18]
    s1a = np.concatenate([u1, u1], axis=0)        # [128, 128] replicated
    s1b = np.concatenate([v1, v1], axis=0)
    b3p = np.zeros((HID, 1), np.float32)
    b3full = (b3 + 0.5 * W3.sum(axis=1)).astype(np.float32)
    for g in range(NPT):
        b3p[32 * g:32 * g + NACT, 0] = b3full
    shared = {
        "s1a": s1a, "s1b": s1b,
        "s2a": np.ascontiguousarray(u2), "s2b": np.ascontiguousarray(v2),
        "s3a": np.ascontiguousarray(u3), "s3b": np.ascontiguousarray(v3),
        "b1v": np.ascontiguousarray(b1[:, None]),
        "b2v": np.ascontiguousarray(
            (b2 + 0.5 * W2.sum(axis=1))[:, None].astype(np.float32)),
        "b3v": b3p,
        "be1": np.ascontiguousarray(be1[:, None]),
        "be2": np.ascontiguousarray(be2[:, None]),
        "ident": np.eye(OBS, dtype=np.float32),
    }
    # x planes, transposed to [row, t, b]: row<64 hi plane, row>=64 lo plane
    xhi = x.astype(np.float16)
    xlo = (x - xhi.astype(np.float32)).astype(np.float16)
    xt = np.concatenate(
        [np.transpose(xhi, (2, 1, 0)), np.transpose(xlo, (2, 1, 0))],
        axis=0)                                   # [128, T, B] fp16
    in_maps = []
    for c in range(NCORES):
        m = dict(shared)
        m["xt"] = np.ascontiguousarray(xt[:, :, c * BC:(c + 1) * BC])
        m["h0"] = np.ascontiguousarray(h0[c * BC:(c + 1) * BC])
        in_maps.append(m)
    return in_maps


def kernel(**inputs) -> np.ndarray:
    from concourse.bass_utils import run_bass_kernel_spmd

    nc = build_nc(T_FULL)
    in_maps = _prep_core_inputs(inputs)
    res = run_bass_kernel_spmd(nc, in_maps, core_ids=list(range(NCORES)))
    # yraw[ch, g, a, s, b] -> y[b, ch*TC + g*SPT + s, a]
    outs = []
    for r in res.results:
        yr = r["yraw"]        # [NCH, NPT, NACT, SPT, BC]
        NCH = yr.shape[0]
        y = np.transpose(yr, (4, 0, 1, 3, 2)).reshape(BC, NCH * TC, NACT)
        outs.append(np.ascontiguousarray(y))
    return np.concatenate(outs, axis=0)


# revision 22
# speedup vs baseline: 1.0059x; 1.0059x over previous
# Trainium2 Bass kernel for nn_DSQN (2-layer LIF spiking MLP, snntorch Leaky
# subtract-reset semantics), batch-data-parallel across 8 NeuronCores.
#
# Math (per core, batch shard of 64):
#   mem_t = be*mem_{t-1} + cur_t - (mem_{t-1} > 1)    [per layer]
#   spk_t = (mem_t > 1)
#   cur1 = x @ W1.T + b1;  cur2 = spk1 @ W2.T + b2;  out = spk2 @ W3.T + b3
#
# v2 design notes (from HW probing):
#  - The per-step LIF recurrence is the critical path: one fused custom DVE
#    op per timestep ([128 hid, 2 layers, 64 batch]) runs at ~290ns/step when
#    its inputs are staged a full chunk ahead (vs ~446ns with same-iteration
#    staging).  The pipeline below computes cur1 for chunk it+1 while the
#    chain runs chunk it, so every chain step's operands are a chunk old.
#  - All matmuls use exact fp16 two-plane splits (probe: fp16 matmul is
#    bit-exact for fp16 inputs, single-pass ~216-375ns/512cols vs fp32's
#    2-pass ~1184ns).  W = U + V with U=fp16(W), V=fp16(W-U) carries 22
#    mantissa bits; x = A + B likewise.  L1 stacks the two x-planes on the
#    PE contraction axis (obs=64, so [A;B] fills 128 rows) with replicated
#    [U;U],[V;V] stationaries - 2 passes/psum tile, PSUM-accumulated.
#  - L3 packs its four [18,512] psum tiles into one bank at partition
#    offsets 0/32/64/96, so one scalar copy (with per-partition bias) moves
#    the whole chunk's output.
#  - Engine budget per 32-step chunk: DVE = 32 LIF + 1 spike-extract;
#    Scalar = 9 psum copies + 1 Sign; Tensor = 24 matmuls; GPSIMD unused
#    (probe: ~25x slower than DVE for bulk elementwise).

import numpy as np

import concourse.bacc as bacc
import concourse.bass as bass
import concourse.mybir as mybir
from concourse.tile import TileContext

F32 = mybir.dt.float32
F16 = mybir.dt.float16
OP = mybir.AluOpType
AF = mybir.ActivationFunctionType

B_TOTAL, T_FULL, OBS, HID, NACT = 512, 2048, 64, 128, 18
NCORES = 8
BC = B_TOTAL // NCORES      # 64 batch per core
TC = 32                     # time steps per chunk
SPT = 8                     # steps per psum tile (512 f32 columns)
NPT = TC // SPT             # 4 psum tiles per chunk


def _register_lif_op():
    """Register the fused LIF-step custom DVE op (runtime, idempotent).

    out[p, s, k] = in0[p, s, k] * (s0[p] if s == 0 else s1[p])
                   + in1[p, s*BC + k] - (in0[p, s, k] > 1.0)
    """
    import concourse.dve_ops as dve_ops
    for o in dve_ops.OPS:
        if o.name == "LIF_STEP_ANT":
            return o
    from concourse.dve_spec import (
        Spec, Src0, Src1, C0, C1, Zero, One, SubIdx, select, eq, lower,
        _has_src1)
    from concourse.dve_uop import DveOpSpec

    def _ref(in0, in1, s0, s1, imm2):
        m = np.asarray(in0, np.float32)
        c = np.asarray(in1, np.float32).reshape(m.shape)
        sc = (s0, s1)
        be = np.stack(
            [np.broadcast_to(
                np.reshape(np.asarray(sc[i], np.float32), (-1, 1)),
                (m.shape[0], m.shape[2]))
             for i in range(m.shape[1])], axis=1)
        r = (m > np.float32(1.0)).astype(np.float32)
        return (m * be + c) - r

    body = Src0 * select(eq(SubIdx, Zero), C0, C1) + Src1 - (Src0 > One)
    spec = Spec(body=body, reference=_ref)
    row = dve_ops._CUSTOM_DVE_ROW_BASE + len(dve_ops.OPS)
    shas = {}
    for ver in ("v3", "v4"):
        try:
            s = DveOpSpec(name="LIF_STEP_ANT", opcode=row,
                          uops=lower(spec, ver=ver), rd1_en=_has_src1(spec))
            shas[ver] = s.sha(ver)
        except Exception:
            pass
    op = dve_ops.DveOp("LIF_STEP_ANT", spec, subdim=True, uops_sha=shas)
    dve_ops.OPS.append(op)
    dve_ops.CUSTOM_DVE_SPECS[op.name] = op.spec
    dve_ops._SUB_OPCODE_FOR_NAME[op.name] = row
    return op


def _register_lif1_op():
    # Single-layer LIF op: single-FSM-state (no subdim/scan/select), so its
    # fixed cost is far lower than the fused two-layer op (dur 280 vs 410ns)
    # and the two per-layer ops of a step are mutually independent, so they
    # overlap on the DVE: 196ns/op cadence = 392ns/step vs 446 fused.
    import concourse.dve_ops as dve_ops
    from concourse.dve_spec import Spec, Src0, Src1, C0, One, lower, _has_src1
    from concourse.dve_uop import DveOpSpec
    for o in dve_ops.OPS:
        if o.name == "LIF1_ANT":
            return o

    def _ref(in0, in1, s0, s1, imm2):
        m = np.asarray(in0, np.float32)
        c = np.asarray(in1, np.float32).reshape(m.shape)
        be = np.reshape(np.asarray(s0, np.float32), (-1, 1))
        return (m * be + c) - (m > np.float32(1.0)).astype(np.float32)

    body = Src0 * C0 + Src1 - (Src0 > One)
    spec = Spec(body=body, reference=_ref)
    row = dve_ops._CUSTOM_DVE_ROW_BASE + len(dve_ops.OPS)
    shas = {}
    for ver in ("v3", "v4"):
        try:
            s = DveOpSpec(name="LIF1_ANT", opcode=row,
                          uops=lower(spec, ver=ver), rd1_en=_has_src1(spec))
            shas[ver] = s.sha(ver)
        except Exception:
            pass
    op = dve_ops.DveOp("LIF1_ANT", spec, subdim=False, uops_sha=shas)
    dve_ops.OPS.append(op)
    dve_ops.CUSTOM_DVE_SPECS[op.name] = op.spec
    dve_ops._SUB_OPCODE_FOR_NAME[op.name] = row
    return op


def build_nc(T=T_FULL):
    NCH = T // TC
    lif_op = _register_lif_op()
    lif1_op = _register_lif1_op()
    nc = bacc.Bacc()

    # x planes: rows 0-63 = fp16 hi plane of obs o, rows 64-127 = fp16 lo
    xt_d = nc.dram_tensor("xt", [HID, T, BC], F16, kind="ExternalInput")
    h0_d = nc.dram_tensor("h0", [BC, HID, 2], F32, kind="ExternalInput")
    # stationaries (fp16 two-plane splits, pre-stacked on host)
    s1a_d = nc.dram_tensor("s1a", [HID, HID], F16, kind="ExternalInput")
    s1b_d = nc.dram_tensor("s1b", [HID, HID], F16, kind="ExternalInput")
    s2a_d = nc.dram_tensor("s2a", [HID, HID], F16, kind="ExternalInput")
    s2b_d = nc.dram_tensor("s2b", [HID, HID], F16, kind="ExternalInput")
    s3a_d = nc.dram_tensor("s3a", [HID, NACT], F16, kind="ExternalInput")
    s3b_d = nc.dram_tensor("s3b", [HID, NACT], F16, kind="ExternalInput")
    b1_d = nc.dram_tensor("b1v", [HID, 1], F32, kind="ExternalInput")
    b2_d = nc.dram_tensor("b2v", [HID, 1], F32, kind="ExternalInput")
    b3_d = nc.dram_tensor("b3v", [HID, 1], F32, kind="ExternalInput")
    be1_d = nc.dram_tensor("be1", [HID, 1], F32, kind="ExternalInput")
    be2_d = nc.dram_tensor("be2", [HID, 1], F32, kind="ExternalInput")
    id_d = nc.dram_tensor("ident", [OBS, OBS], F32, kind="ExternalInput")
    # raw packed output: per chunk the [128, 512] ob tile (4 blocks of 18
    # used partitions at offsets 0/32/64/96); host unpacks
    y_d = nc.dram_tensor("yraw", [T // TC, NPT, NACT, SPT, BC], F32,
                         kind="ExternalOutput")

    with TileContext(nc) as tc:
        with (
            tc.tile_pool(name="const", bufs=1) as cpool,
            tc.tile_pool(name="state", bufs=1) as spool,
            tc.tile_pool(name="xt", bufs=3) as xt_pool,
            tc.tile_pool(name="mb", bufs=3) as mb_pool,
            tc.tile_pool(name="cb", bufs=4) as cb_pool,
            tc.tile_pool(name="spk1", bufs=2) as spk1_pool,
            tc.tile_pool(name="spk2", bufs=2) as spk2_pool,
            tc.tile_pool(name="ob", bufs=2) as ob_pool,
            tc.tile_pool(name="pc1", bufs=2, space=bass.MemorySpace.PSUM) as pc1_pool,
            tc.tile_pool(name="pc2", bufs=2, space=bass.MemorySpace.PSUM) as pc2_pool,
            tc.tile_pool(name="pout", bufs=2, space=bass.MemorySpace.PSUM) as po_pool,
        ):
            # ---- constants ----
            s1a = cpool.tile([HID, HID], F16, tag="s1a")
            s1b = cpool.tile([HID, HID], F16, tag="s1b")
            s2a = cpool.tile([HID, HID], F16, tag="s2a")
            s2b = cpool.tile([HID, HID], F16, tag="s2b")
            s3a = cpool.tile([HID, NACT], F16, tag="s3a")
            s3b = cpool.tile([HID, NACT], F16, tag="s3b")
            b1v = cpool.tile([HID, 1], F32, tag="b1v")
            b2v = cpool.tile([HID, 1], F32, tag="b2v")
            b3v = cpool.tile([HID, 1], F32, tag="b3v")
            be1 = cpool.tile([HID, 1], F32, tag="be1")
            be2 = cpool.tile([HID, 1], F32, tag="be2")
            ident = cpool.tile([OBS, OBS], F32, tag="ident")
            negone = cpool.tile([HID, 1], F32, tag="negone")
            nc.gpsimd.memset(negone[:], -1.0)

            mbufs, cbufs, spk1bufs, xtbufs = {}, {}, {}, {}

            def get_cb(n):
                if n not in cbufs:
                    cbufs[n] = cb_pool.tile([HID, TC, 2, BC], F32, tag="cb",
                                            name=f"cb{n}")
                return cbufs[n]

            def load_xt(n):
                if n not in xtbufs:
                    xt = xt_pool.tile([HID, TC, BC], F16, tag="xt",
                                      name=f"xt{n}")
                    nc.sync.dma_start(xt[:], xt_d[:, n * TC:(n + 1) * TC, :])
                    xtbufs[n] = xt
                return xtbufs[n]

            # critical-path DMAs first: everything the first chain chunk
            # needs (xt[0], h0, ident, L1 weights, chain constants) issues
            # before the bulk L2/L3 weights -- the sync queue is serial and
            # the first LIF waits on this whole path
            load_xt(0)
            h0ns = []
            for ch in range(2):
                h0n = spool.tile([BC, HID], F32, tag=f"h0n{ch}")
                nc.sync.dma_start(h0n[:], h0_d[:, :, ch])
                h0ns.append(h0n)
            for t_, d_ in ((ident, id_d), (s1a, s1a_d), (s1b, s1b_d),
                           (b1v, b1_d), (be1, be1_d), (be2, be2_d)):
                nc.sync.dma_start(t_[:], d_[:])

            # ---- initial membranes: [128 hid, 2 layer, 64 batch] ----
            minit = spool.tile([HID, 2, BC], F32, tag="minit")
            for ch in range(2):
                pt = pc1_pool.tile([HID, BC], F32, tag="c1ps", name=f"init{ch}")
                nc.tensor.transpose(pt[:], h0ns[ch][:], ident[:])
                nc.scalar.activation(minit[:, ch, :], pt[:], AF.Copy)

            # remaining weights (needed only from stage C / D onwards)
            for t_, d_ in ((b2v, b2_d), (s2a, s2a_d), (s2b, s2b_d),
                           (s3a, s3a_d), (s3b, s3b_d), (b3v, b3_d)):
                nc.sync.dma_start(t_[:], d_[:])

            for it in range(-1, NCH + 2):
                l1p = it + 1 if 0 <= it + 1 < NCH else None   # L1 prefetch
                l1 = it if 0 <= it < NCH else None            # layer1 of chain
                l2 = it - 2 if 0 <= it - 2 < NCH else None    # layer2 of chain
                jc2 = it - 1 if 0 <= it - 1 < NCH else None   # L2 mm source

                # ---- xt DMA two chunks ahead ----
                if 0 <= it + 2 < NCH:
                    load_xt(it + 2)

                # ---- stage A: cur1 for chunk it+1 (one chunk ahead) ----
                if l1p is not None:
                    xt = load_xt(l1p)
                    c1b = get_cb(l1p)
                    for g in range(NPT):
                        pc = pc1_pool.tile([HID, SPT, BC], F32, tag="c1ps")
                        nc.tensor.matmul(
                            pc[:], s1a[:], xt[:, g * SPT:(g + 1) * SPT, :],
                            start=True, stop=False)
                        nc.tensor.matmul(
                            pc[:], s1b[:], xt[:, g * SPT:(g + 1) * SPT, :],
                            start=False, stop=True)
                        nc.scalar.activation(
                            c1b[:, g * SPT:(g + 1) * SPT, 0, :], pc[:],
                            AF.Identity, bias=b1v[:])

                # ---- stage B: fused LIF chain for chunk it ----
                if l1 is not None or l2 is not None:
                    mb = mb_pool.tile([HID, TC, 2, BC], F32, tag="mb",
                                      name=f"mb{it}")
                    mbufs[it] = mb
                    cb = get_cb(it)
                    for s in range(TC):
                        if s == 0:
                            pm = mbufs.get(it - 1)
                            prev = minit[:] if pm is None else pm[:, TC - 1]
                        else:
                            prev = mb[:, s - 1]
                        if l1 is not None:
                            nc.vector._custom_dve(
                                lif1_op, out=mb[:, s, 0, :],
                                in0=prev[:, 0, :], in1=cb[:, s, 0, :],
                                s0=be1[:], s1=0.0)
                        if l2 is not None:
                            nc.vector._custom_dve(
                                lif1_op, out=mb[:, s, 1, :],
                                in0=prev[:, 1, :], in1=cb[:, s, 1, :],
                                s0=be2[:], s1=0.0)
                    if it == 1:  # seed layer-2 initial membrane for handoff
                        nc.vector.tensor_copy(mb[:, TC - 1, 1, :],
                                              minit[:, 1, :])
                    # spk1 of chunk it: sign(m-1) in {-1,+1} fp16 on the
                    # Scalar engine (keeps the DVE a pure LIF chain); W2 is
                    # halved on host and b2 absorbs +0.5*rowsum(W2)
                    if l1 is not None:
                        spk1 = spk1_pool.tile([HID, TC, BC], F16, tag="spk1",
                                              name=f"s1_{it}")
                        spk1bufs[it] = spk1
                        nc.scalar.activation(
                            spk1[:], mb[:, :, 0, :], AF.Sign, bias=negone[:])

                # ---- stage C: cur2 for chunk jc2 from spk1[jc2] ----
                if jc2 is not None:
                    c2dst = get_cb(jc2 + 2)
                    sm = spk1bufs[jc2]
                    for g in range(NPT):
                        pc = pc2_pool.tile([HID, SPT, BC], F32, tag="c2ps")
                        nc.tensor.matmul(
                            pc[:], s2a[:], sm[:, g * SPT:(g + 1) * SPT, :],
                            start=True, stop=False)
                        nc.tensor.matmul(
                            pc[:], s2b[:], sm[:, g * SPT:(g + 1) * SPT, :],
                            start=False, stop=True)
                        nc.scalar.activation(
                            c2dst[:, g * SPT:(g + 1) * SPT, 1, :], pc[:],
                            AF.Identity, bias=b2v[:])

                # ---- stage D: out for chunk l2 (sign spikes, packed psum) --
                if l2 is not None:
                    # spk2 as sign(m-1) in {-1,+1} fp16 (Scalar engine);
                    # W3 halved on host, bias3 absorbs +0.5*rowsum(W3)
                    spk2 = spk2_pool.tile([HID, TC, BC], F16, tag="spk2",
                                          name=f"s2_{it}")
                    if it >= NCH:
                        # tail iterations: halve the Sign so extraction (and
                        # the L3 chain behind it) starts mid-chunk instead of
                        # after the final chain op -- shrinks the epilogue
                        for hh in range(2):
                            nc.scalar.activation(
                                spk2[:, hh * (TC // 2):(hh + 1) * (TC // 2)],
                                mbufs[it][:, hh * (TC // 2):
                                          (hh + 1) * (TC // 2), 1, :],
                                AF.Sign, bias=negone[:])
                    else:
                        nc.scalar.activation(
                            spk2[:], mbufs[it][:, :, 1, :], AF.Sign,
                            bias=negone[:])
                    # PE out base partition must be 0/32/64 (quadrant-3 bug):
                    # pack psum tiles 3+1 across two banks
                    po = po_pool.tile([HID, SPT, BC], F32, tag="ops")
                    pob = po_pool.tile([NACT, SPT, BC], F32, tag="opsb")
                    for g in range(NPT):
                        dst = po[32 * g:32 * g + NACT] if g < 3 else pob[:]
                        nc.tensor.matmul(
                            dst, s3a[:], spk2[:, g * SPT:(g + 1) * SPT, :],
                            start=True, stop=False)
                        nc.tensor.matmul(
                            dst, s3b[:], spk2[:, g * SPT:(g + 1) * SPT, :],
                            start=False, stop=True)
                    ob = ob_pool.tile([HID, SPT, BC], F32, tag="ob")
                    obb = ob_pool.tile([NACT, SPT, BC], F32, tag="obb")
                    nc.scalar.activation(ob[:], po[:], AF.Identity,
                                         bias=b3v[:])
                    nc.scalar.activation(obb[:], pob[:], AF.Identity,
                                         bias=b3v[0:NACT])
                    for g in range(3):
                        nc.sync.dma_start(y_d[l2, g],
                                          ob[32 * g:32 * g + NACT])
                    nc.sync.dma_start(y_d[l2, 3], obb[:])

    nc.compile()
    return nc


def _fp16_split(w):
    """w (fp32) -> (hi, lo) fp16 planes with hi+lo capturing 22 bits."""
    hi = w.astype(np.float16)
    lo = (w - hi.astype(np.float32)).astype(np.float16)
    return hi, lo


def _prep_core_inputs(inputs):
    x = np.ascontiguousarray(np.asarray(inputs["state_batch"], np.float32))
    h0 = np.asarray(inputs["hidden_states"], np.float32)[:, 0, :, :]
    W1 = np.asarray(inputs["W1"], np.float32)
    W2 = np.asarray(inputs["W2"], np.float32)
    W3 = np.asarray(inputs["W3"], np.float32)
    b1 = np.asarray(inputs["b1"], np.float32)
    b2 = np.asarray(inputs["b2"], np.float32)
    b3 = np.asarray(inputs["b3"], np.float32)
    be1 = np.clip(np.asarray(inputs["beta1"], np.float32), 0.0, 1.0)
    be2 = np.clip(np.asarray(inputs["beta2"], np.float32), 0.0, 1.0)

    u1, v1 = _fp16_split(W1.T)                    # [64, 128]
    u2, v2 = _fp16_split((0.5 * W2).T)            # [128, 128]
    u3, v3 = _fp16_split((0.5 * W3).T)            # [128, 18]
    s1a = np.concatenate([u1, u1], axis=0)        # [128, 128] replicated
    s1b = np.concatenate([v1, v1], axis=0)
    b3p = np.zeros((HID, 1), np.float32)
    b3full = (b3 + 0.5 * W3.sum(axis=1)).astype(np.float32)
    for g in range(NPT):
        b3p[32 * g:32 * g + NACT, 0] = b3full
    shared = {
        "s1a": s1a, "s1b": s1b,
        "s2a": np.ascontiguousarray(u2), "s2b": np.ascontiguousarray(v2),
        "s3a": np.ascontiguousarray(u3), "s3b": np.ascontiguousarray(v3),
        "b1v": np.ascontiguousarray(b1[:, None]),
        "b2v": np.ascontiguousarray(
            (b2 + 0.5 * W2.sum(axis=1))[:, None].astype(np.float32)),
        "b3v": b3p,
        "be1": np.ascontiguousarray(be1[:, None]),
        "be2": np.ascontiguousarray(be2[:, None]),
        "ident": np.eye(OBS, dtype=np.float32),
    }
    # x planes, transposed to [row, t, b]: row<64 hi plane, row>=64 lo plane
    xhi = x.astype(np.float16)
    xlo = (x - xhi.astype(np.float32)).astype(np.float16)
    xt = np.concatenate(
        [np.transpose(xhi, (2, 1, 0)), np.transpose(xlo, (2, 1, 0))],
        axis=0)                                   # [128, T, B] fp16
    in_maps = []
    for c in range(NCORES):
        m = dict(shared)
        m["xt"] = np.ascontiguousarray(xt[:, :, c * BC:(c + 1) * BC])
        m["h0"] = np.ascontiguousarray(h0[c * BC:(c + 1) * BC])
        in_maps.append(m)
    return in_maps


def kernel(**inputs) -> np.ndarray:
    from concourse.bass_utils import run_bass_kernel_spmd

    nc = build_nc(T_FULL)
    in_maps = _prep_core_inputs(inputs)
    res = run_bass_kernel_spmd(nc, in_maps, core_ids=list(range(NCORES)))
    # yraw[ch, g, a, s, b] -> y[b, ch*TC + g*SPT + s, a]
    outs = []
    for r in res.results:
        yr = r["yraw"]        # [NCH, NPT, NACT, SPT, BC]
        NCH = yr.shape[0]
        y = np.transpose(yr, (4, 0, 1, 3, 2)).reshape(BC, NCH * TC, NACT)
        outs.append(np.ascontiguousarray(y))
    return np.concatenate(outs, axis=0)


# revision 23
# speedup vs baseline: 1.0061x; 1.0002x over previous
# Trainium2 Bass kernel for nn_DSQN (2-layer LIF spiking MLP, snntorch Leaky
# subtract-reset semantics), batch-data-parallel across 8 NeuronCores.
#
# Math (per core, batch shard of 64):
#   mem_t = be*mem_{t-1} + cur_t - (mem_{t-1} > 1)    [per layer]
#   spk_t = (mem_t > 1)
#   cur1 = x @ W1.T + b1;  cur2 = spk1 @ W2.T + b2;  out = spk2 @ W3.T + b3
#
# v2 design notes (from HW probing):
#  - The per-step LIF recurrence is the critical path: one fused custom DVE
#    op per timestep ([128 hid, 2 layers, 64 batch]) runs at ~290ns/step when
#    its inputs are staged a full chunk ahead (vs ~446ns with same-iteration
#    staging).  The pipeline below computes cur1 for chunk it+1 while the
#    chain runs chunk it, so every chain step's operands are a chunk old.
#  - All matmuls use exact fp16 two-plane splits (probe: fp16 matmul is
#    bit-exact for fp16 inputs, single-pass ~216-375ns/512cols vs fp32's
#    2-pass ~1184ns).  W = U + V with U=fp16(W), V=fp16(W-U) carries 22
#    mantissa bits; x = A + B likewise.  L1 stacks the two x-planes on the
#    PE contraction axis (obs=64, so [A;B] fills 128 rows) with replicated
#    [U;U],[V;V] stationaries - 2 passes/psum tile, PSUM-accumulated.
#  - L3 packs its four [18,512] psum tiles into one bank at partition
#    offsets 0/32/64/96, so one scalar copy (with per-partition bias) moves
#    the whole chunk's output.
#  - Engine budget per 32-step chunk: DVE = 32 LIF + 1 spike-extract;
#    Scalar = 9 psum copies + 1 Sign; Tensor = 24 matmuls; GPSIMD unused
#    (probe: ~25x slower than DVE for bulk elementwise).

import numpy as np

import concourse.bacc as bacc
import concourse.bass as bass
import concourse.mybir as mybir
from concourse.tile import TileContext

F32 = mybir.dt.float32
F16 = mybir.dt.float16
OP = mybir.AluOpType
AF = mybir.ActivationFunctionType

B_TOTAL, T_FULL, OBS, HID, NACT = 512, 2048, 64, 128, 18
NCORES = 8
BC = B_TOTAL // NCORES      # 64 batch per core
TC = 32                     # time steps per chunk
SPT = 8                     # steps per psum tile (512 f32 columns)
NPT = TC // SPT             # 4 psum tiles per chunk


def _register_lif_op():
    """Register the fused LIF-step custom DVE op (runtime, idempotent).

    out[p, s, k] = in0[p, s, k] * (s0[p] if s == 0 else s1[p])
                   + in1[p, s*BC + k] - (in0[p, s, k] > 1.0)
    """
    import concourse.dve_ops as dve_ops
    for o in dve_ops.OPS:
        if o.name == "LIF_STEP_ANT":
            return o
    from concourse.dve_spec import (
        Spec, Src0, Src1, C0, C1, Zero, One, SubIdx, select, eq, lower,
        _has_src1)
    from concourse.dve_uop import DveOpSpec

    def _ref(in0, in1, s0, s1, imm2):
        m = np.asarray(in0, np.float32)
        c = np.asarray(in1, np.float32).reshape(m.shape)
        sc = (s0, s1)
        be = np.stack(
            [np.broadcast_to(
                np.reshape(np.asarray(sc[i], np.float32), (-1, 1)),
                (m.shape[0], m.shape[2]))
             for i in range(m.shape[1])], axis=1)
        r = (m > np.float32(1.0)).astype(np.float32)
        return (m * be + c) - r

    body = Src0 * select(eq(SubIdx, Zero), C0, C1) + Src1 - (Src0 > One)
    spec = Spec(body=body, reference=_ref)
    row = dve_ops._CUSTOM_DVE_ROW_BASE + len(dve_ops.OPS)
    shas = {}
    for ver in ("v3", "v4"):
        try:
            s = DveOpSpec(name="LIF_STEP_ANT", opcode=row,
                          uops=lower(spec, ver=ver), rd1_en=_has_src1(spec))
            shas[ver] = s.sha(ver)
        except Exception:
            pass
    op = dve_ops.DveOp("LIF_STEP_ANT", spec, subdim=True, uops_sha=shas)
    dve_ops.OPS.append(op)
    dve_ops.CUSTOM_DVE_SPECS[op.name] = op.spec
    dve_ops._SUB_OPCODE_FOR_NAME[op.name] = row
    return op


def _register_lif1_op():
    # Single-layer LIF op: single-FSM-state (no subdim/scan/select), so its
    # fixed cost is far lower than the fused two-layer op (dur 280 vs 410ns)
    # and the two per-layer ops of a step are mutually independent, so they
    # overlap on the DVE: 196ns/op cadence = 392ns/step vs 446 fused.
    import concourse.dve_ops as dve_ops
    from concourse.dve_spec import Spec, Src0, Src1, C0, One, lower, _has_src1
    from concourse.dve_uop import DveOpSpec
    for o in dve_ops.OPS:
        if o.name == "LIF1_ANT":
            return o

    def _ref(in0, in1, s0, s1, imm2):
        m = np.asarray(in0, np.float32)
        c = np.asarray(in1, np.float32).reshape(m.shape)
        be = np.reshape(np.asarray(s0, np.float32), (-1, 1))
        return (m * be + c) - (m > np.float32(1.0)).astype(np.float32)

    body = Src0 * C0 + Src1 - (Src0 > One)
    spec = Spec(body=body, reference=_ref)
    row = dve_ops._CUSTOM_DVE_ROW_BASE + len(dve_ops.OPS)
    shas = {}
    for ver in ("v3", "v4"):
        try:
            s = DveOpSpec(name="LIF1_ANT", opcode=row,
                          uops=lower(spec, ver=ver), rd1_en=_has_src1(spec))
            shas[ver] = s.sha(ver)
        except Exception:
            pass
    op = dve_ops.DveOp("LIF1_ANT", spec, subdim=False, uops_sha=shas)
    dve_ops.OPS.append(op)
    dve_ops.CUSTOM_DVE_SPECS[op.name] = op.spec
    dve_ops._SUB_OPCODE_FOR_NAME[op.name] = row
    return op


def build_nc(T=T_FULL):
    NCH = T // TC
    lif_op = _register_lif_op()
    lif1_op = _register_lif1_op()
    nc = bacc.Bacc()

    # x planes: rows 0-63 = fp16 hi plane of obs o, rows 64-127 = fp16 lo
    xt_d = nc.dram_tensor("xt", [HID, T, BC], F16, kind="ExternalInput")
    h0_d = nc.dram_tensor("h0", [BC, HID, 2], F32, kind="ExternalInput")
    # stationaries (fp16 two-plane splits, pre-stacked on host)
    s1a_d = nc.dram_tensor("s1a", [HID, HID], F16, kind="ExternalInput")
    s1b_d = nc.dram_tensor("s1b", [HID, HID], F16, kind="ExternalInput")
    s2a_d = nc.dram_tensor("s2a", [HID, HID], F16, kind="ExternalInput")
    s2b_d = nc.dram_tensor("s2b", [HID, HID], F16, kind="ExternalInput")
    s3a_d = nc.dram_tensor("s3a", [HID, NACT], F16, kind="ExternalInput")
    s3b_d = nc.dram_tensor("s3b", [HID, NACT], F16, kind="ExternalInput")
    b1_d = nc.dram_tensor("b1v", [HID, 1], F32, kind="ExternalInput")
    b2_d = nc.dram_tensor("b2v", [HID, 1], F32, kind="ExternalInput")
    b3_d = nc.dram_tensor("b3v", [HID, 1], F32, kind="ExternalInput")
    be1_d = nc.dram_tensor("be1", [HID, 1], F32, kind="ExternalInput")
    be2_d = nc.dram_tensor("be2", [HID, 1], F32, kind="ExternalInput")
    id_d = nc.dram_tensor("ident", [OBS, OBS], F32, kind="ExternalInput")
    # raw packed output: per chunk the [128, 512] ob tile (4 blocks of 18
    # used partitions at offsets 0/32/64/96); host unpacks
    y_d = nc.dram_tensor("yraw", [T // TC, NPT, NACT, SPT, BC], F32,
                         kind="ExternalOutput")

    with TileContext(nc) as tc:
        with (
            tc.tile_pool(name="const", bufs=1) as cpool,
            tc.tile_pool(name="state", bufs=1) as spool,
            tc.tile_pool(name="xt", bufs=3) as xt_pool,
            tc.tile_pool(name="mb", bufs=3) as mb_pool,
            tc.tile_pool(name="cb", bufs=4) as cb_pool,
            tc.tile_pool(name="spk1", bufs=2) as spk1_pool,
            tc.tile_pool(name="spk2", bufs=2) as spk2_pool,
            tc.tile_pool(name="ob", bufs=2) as ob_pool,
            tc.tile_pool(name="pc1", bufs=2, space=bass.MemorySpace.PSUM) as pc1_pool,
            tc.tile_pool(name="pc2", bufs=2, space=bass.MemorySpace.PSUM) as pc2_pool,
            tc.tile_pool(name="pout", bufs=2, space=bass.MemorySpace.PSUM) as po_pool,
        ):
            # ---- constants ----
            s1a = cpool.tile([HID, HID], F16, tag="s1a")
            s1b = cpool.tile([HID, HID], F16, tag="s1b")
            s2a = cpool.tile([HID, HID], F16, tag="s2a")
            s2b = cpool.tile([HID, HID], F16, tag="s2b")
            s3a = cpool.tile([HID, NACT], F16, tag="s3a")
            s3b = cpool.tile([HID, NACT], F16, tag="s3b")
            b1v = cpool.tile([HID, 1], F32, tag="b1v")
            b2v = cpool.tile([HID, 1], F32, tag="b2v")
            b3v = cpool.tile([HID, 1], F32, tag="b3v")
            be1 = cpool.tile([HID, 1], F32, tag="be1")
            be2 = cpool.tile([HID, 1], F32, tag="be2")
            ident = cpool.tile([OBS, OBS], F32, tag="ident")
            negone = cpool.tile([HID, 1], F32, tag="negone")
            nc.gpsimd.memset(negone[:], -1.0)

            mbufs, cbufs, spk1bufs, xtbufs = {}, {}, {}, {}

            def get_cb(n):
                if n not in cbufs:
                    cbufs[n] = cb_pool.tile([HID, TC, 2, BC], F32, tag="cb",
                                            name=f"cb{n}")
                return cbufs[n]

            def load_xt(n):
                if n not in xtbufs:
                    xt = xt_pool.tile([HID, TC, BC], F16, tag="xt",
                                      name=f"xt{n}")
                    nc.sync.dma_start(xt[:], xt_d[:, n * TC:(n + 1) * TC, :])
                    xtbufs[n] = xt
                return xtbufs[n]

            # critical-path DMAs first: everything the first chain chunk
            # needs (xt[0], h0, ident, L1 weights, chain constants) issues
            # before the bulk L2/L3 weights -- the sync queue is serial and
            # the first LIF waits on this whole path
            load_xt(0)
            h0ns = []
            for ch in range(2):
                h0n = spool.tile([BC, HID], F32, tag=f"h0n{ch}")
                nc.sync.dma_start(h0n[:], h0_d[:, :, ch])
                h0ns.append(h0n)
            for t_, d_ in ((ident, id_d), (s1a, s1a_d), (s1b, s1b_d),
                           (b1v, b1_d), (be1, be1_d), (be2, be2_d)):
                nc.sync.dma_start(t_[:], d_[:])

            # ---- initial membranes: [128 hid, 2 layer, 64 batch] ----
            minit = spool.tile([HID, 2, BC], F32, tag="minit")
            for ch in range(2):
                pt = pc1_pool.tile([HID, BC], F32, tag="c1ps", name=f"init{ch}")
                nc.tensor.transpose(pt[:], h0ns[ch][:], ident[:])
                nc.scalar.activation(minit[:, ch, :], pt[:], AF.Copy)

            # remaining weights (needed only from stage C / D onwards)
            for t_, d_ in ((b2v, b2_d), (s2a, s2a_d), (s2b, s2b_d),
                           (s3a, s3a_d), (s3b, s3b_d), (b3v, b3_d)):
                nc.sync.dma_start(t_[:], d_[:])

            for it in range(-1, NCH + 2):
                l1p = it + 1 if 0 <= it + 1 < NCH else None   # L1 prefetch
                l1 = it if 0 <= it < NCH else None            # layer1 of chain
                l2 = it - 2 if 0 <= it - 2 < NCH else None    # layer2 of chain
                jc2 = it - 1 if 0 <= it - 1 < NCH else None   # L2 mm source

                # ---- xt DMA two chunks ahead ----
                if 0 <= it + 2 < NCH:
                    load_xt(it + 2)

                # ---- stage A: cur1 for chunk it+1 (one chunk ahead) ----
                if l1p is not None:
                    xt = load_xt(l1p)
                    c1b = get_cb(l1p)
                    for g in range(NPT):
                        pc = pc1_pool.tile([HID, SPT, BC], F32, tag="c1ps")
                        nc.tensor.matmul(
                            pc[:], s1a[:], xt[:, g * SPT:(g + 1) * SPT, :],
                            start=True, stop=False)
                        nc.tensor.matmul(
                            pc[:], s1b[:], xt[:, g * SPT:(g + 1) * SPT, :],
                            start=False, stop=True)
                        nc.scalar.activation(
                            c1b[:, g * SPT:(g + 1) * SPT, 0, :], pc[:],
                            AF.Identity, bias=b1v[:])

                # ---- stage B: fused LIF chain for chunk it ----
                if l1 is not None or l2 is not None:
                    mb = mb_pool.tile([HID, TC, 2, BC], F32, tag="mb",
                                      name=f"mb{it}")
                    mbufs[it] = mb
                    cb = get_cb(it)
                    for s in range(TC):
                        if s == 0:
                            pm = mbufs.get(it - 1)
                            prev = minit[:] if pm is None else pm[:, TC - 1]
                        else:
                            prev = mb[:, s - 1]
                        if l1 is not None:
                            nc.vector._custom_dve(
                                lif1_op, out=mb[:, s, 0, :],
                                in0=prev[:, 0, :], in1=cb[:, s, 0, :],
                                s0=be1[:], s1=0.0)
                        if l2 is not None:
                            nc.vector._custom_dve(
                                lif1_op, out=mb[:, s, 1, :],
                                in0=prev[:, 1, :], in1=cb[:, s, 1, :],
                                s0=be2[:], s1=0.0)
                    if it == 1:  # seed layer-2 initial membrane for handoff
                        nc.vector.tensor_copy(mb[:, TC - 1, 1, :],
                                              minit[:, 1, :])
                    # spk1 of chunk it: sign(m-1) in {-1,+1} fp16 on the
                    # Scalar engine (keeps the DVE a pure LIF chain); W2 is
                    # halved on host and b2 absorbs +0.5*rowsum(W2)
                    if l1 is not None:
                        spk1 = spk1_pool.tile([HID, TC, BC], F16, tag="spk1",
                                              name=f"s1_{it}")
                        spk1bufs[it] = spk1
                        nc.scalar.activation(
                            spk1[:], mb[:, :, 0, :], AF.Sign, bias=negone[:])

                # ---- stage C: cur2 for chunk jc2 from spk1[jc2] ----
                if jc2 is not None:
                    c2dst = get_cb(jc2 + 2)
                    sm = spk1bufs[jc2]
                    for g in range(NPT):
                        pc = pc2_pool.tile([HID, SPT, BC], F32, tag="c2ps")
                        nc.tensor.matmul(
                            pc[:], s2a[:], sm[:, g * SPT:(g + 1) * SPT, :],
                            start=True, stop=False)
                        nc.tensor.matmul(
                            pc[:], s2b[:], sm[:, g * SPT:(g + 1) * SPT, :],
                            start=False, stop=True)
                        nc.scalar.activation(
                            c2dst[:, g * SPT:(g + 1) * SPT, 1, :], pc[:],
                            AF.Identity, bias=b2v[:])

                # ---- stage D: out for chunk l2 (sign spikes, packed psum) --
                if l2 is not None:
                    # spk2 as sign(m-1) in {-1,+1} fp16 (Scalar engine);
                    # W3 halved on host, bias3 absorbs +0.5*rowsum(W3)
                    spk2 = spk2_pool.tile([HID, TC, BC], F16, tag="spk2",
                                          name=f"s2_{it}")
                    nc.scalar.activation(
                        spk2[:], mbufs[it][:, :, 1, :], AF.Sign,
                        bias=negone[:])
                    # PE out base partition must be 0/32/64 (quadrant-3 bug):
                    # pack psum tiles 3+1 across two banks
                    po = po_pool.tile([HID, SPT, BC], F32, tag="ops")
                    pob = po_pool.tile([NACT, SPT, BC], F32, tag="opsb")
                    for g in range(NPT):
                        dst = po[32 * g:32 * g + NACT] if g < 3 else pob[:]
                        nc.tensor.matmul(
                            dst, s3a[:], spk2[:, g * SPT:(g + 1) * SPT, :],
                            start=True, stop=False)
                        nc.tensor.matmul(
                            dst, s3b[:], spk2[:, g * SPT:(g + 1) * SPT, :],
                            start=False, stop=True)
                    ob = ob_pool.tile([HID, SPT, BC], F32, tag="ob")
                    obb = ob_pool.tile([NACT, SPT, BC], F32, tag="obb")
                    nc.scalar.activation(ob[:], po[:], AF.Identity,
                                         bias=b3v[:])
                    nc.scalar.activation(obb[:], pob[:], AF.Identity,
                                         bias=b3v[0:NACT])
                    for g in range(3):
                        nc.sync.dma_start(y_d[l2, g],
                                          ob[32 * g:32 * g + NACT])
                    nc.sync.dma_start(y_d[l2, 3], obb[:])

    nc.compile()
    return nc


def _fp16_split(w):
    """w (fp32) -> (hi, lo) fp16 planes with hi+lo capturing 22 bits."""
    hi = w.astype(np.float16)
    lo = (w - hi.astype(np.float32)).astype(np.float16)
    return hi, lo


def _prep_core_inputs(inputs):
    x = np.ascontiguousarray(np.asarray(inputs["state_batch"], np.float32))
    h0 = np.asarray(inputs["hidden_states"], np.float32)[:, 0, :, :]
    W1 = np.asarray(inputs["W1"], np.float32)
    W2 = np.asarray(inputs["W2"], np.float32)
    W3 = np.asarray(inputs["W3"], np.float32)
    b1 = np.asarray(inputs["b1"], np.float32)
    b2 = np.asarray(inputs["b2"], np.float32)
    b3 = np.asarray(inputs["b3"], np.float32)
    be1 = np.clip(np.asarray(inputs["beta1"], np.float32), 0.0, 1.0)
    be2 = np.clip(np.asarray(inputs["beta2"], np.float32), 0.0, 1.0)

    u1, v1 = _fp16_split(W1.T)                    # [64, 128]
    u2, v2 = _fp16_split((0.5 * W2).T)            # [128, 128]
    u3, v3 = _fp16_split((0.5 * W3).T)            # [128, 18]
    s1a = np.concatenate([u1, u1], axis=0)        # [128, 128] replicated
    s1b = np.concatenate([v1, v1], axis=0)
    b3p = np.zeros((HID, 1), np.float32)
    b3full = (b3 + 0.5 * W3.sum(axis=1)).astype(np.float32)
    for g in range(NPT):
        b3p[32 * g:32 * g + NACT, 0] = b3full
    shared = {
        "s1a": s1a, "s1b": s1b,
        "s2a": np.ascontiguousarray(u2), "s2b": np.ascontiguousarray(v2),
        "s3a": np.ascontiguousarray(u3), "s3b": np.ascontiguousarray(v3),
        "b1v": np.ascontiguousarray(b1[:, None]),
        "b2v": np.ascontiguousarray(
            (b2 + 0.5 * W2.sum(axis=1))[:, None].astype(np.float32)),
        "b3v": b3p,
        "be1": np.ascontiguousarray(be1[:, None]),
        "be2": np.ascontiguousarray(be2[:, None]),
        "ident": np.eye(OBS, dtype=np.float32),
    }
    # x planes, transposed to [row, t, b]: row<64 hi plane, row>=64 lo plane
    xhi = x.astype(np.float16)
    xlo = (x - xhi.astype(np.float32)).astype(np.float16)
    xt = np.concatenate(
        [np.transpose(xhi, (2, 1, 0)), np.transpose(xlo, (2, 1, 0))],
        axis=0)                                   # [128, T, B] fp16
    in_maps = []
    for c in range(NCORES):
        m = dict(shared)
        m["xt"] = np.ascontiguousarray(xt[:, :, c * BC:(c + 1) * BC])
        m["h0"] = np.ascontiguousarray(h0[c * BC:(c + 1) * BC])
        in_maps.append(m)
    return in_maps


def kernel(**inputs) -> np.ndarray:
    from concourse.bass_utils import run_bass_kernel_spmd

    nc = build_nc(T_FULL)
    in_maps = _prep_core_inputs(inputs)
    res = run_bass_kernel_spmd(nc, in_maps, core_ids=list(range(NCORES)))
    # yraw[ch, g, a, s, b] -> y[b, ch*TC + g*SPT + s, a]
    outs = []
    for r in res.results:
        yr = r["yraw"]        # [NCH, NPT, NACT, SPT, BC]
        NCH = yr.shape[0]
        y = np.transpose(yr, (4, 0, 1, 3, 2)).reshape(BC, NCH * TC, NACT)
        outs.append(np.ascontiguousarray(y))
    return np.concatenate(outs, axis=0)
